# revision 39
# baseline (speedup 1.0000x reference)
"""Port-Hamiltonian model forward pass (dstate/dt) as a Bass/Tile kernel on
8 TRN2 NeuronCores, pure data-parallel over the batch.

Math (per sample, feature-major / transposed layout on chip):
    z1T = W1.T sT + b1                  [512, n]
    h1T = softplus(z1T) = Ln(Exp(z1T)+1)  (fp8 out on the Ln)
    z2T = W2.T h1T                      (fp8 DoubleRow matmuls, x W2S)
    t2  = tanh((z2T + b2)/2)            (fp8 out; sigma(z2) = (1+t2)/2)
    uT  = (0.5 W2 w3).T t2 + c          c_j = 0.5 sum_i W2[j,i] w3[i]
    g1T = (uT + c) * sigmoid(z1T)       one DVE scalar_tensor_tensor per chunk
    outT = (M @ W1) g1T + [0; G_u]      G_u = action_emb @ Gw + Gb on host
  where M = [[0, 1], [-1, -damping]].

Two ACT table phases per core (exp/ln set, then sigmoid/tanh set); the
big H x H matmuls (z2 forward, u backward) run as fp8e4 DoubleRow pairs
(K=256 per matmul, 2x PE throughput).  Weight scales W2S/WUS keep fp8
operands in the normal range; the tanh centering removes the sigma-mean
bulk from the backward operand so fp8 quantization noise halves.

Structure per core (16384 samples, 32 slices of 512):
  - phase A (exp/ln table): z1 (4 concurrent row-tiled K=7 matmuls)
    -> exp -> ln(x+1) -> h1 fp8 -> z2 (8 DoubleRow MMs) -> z2 stored
    bf16 in SBUF by DVE.  PE pipelined one slice behind ACT.
  - phase B (sigmoid table): t2 = tanh(z2s/2) fp8 (scale rides an AP
    written after the last z2s store, fencing the table switch), z1
    recomputed, sg1 = sigmoid(z1), u (8 DoubleRow MMs), g1 = (u+c)*sg1
    fused on DVE, out = w1ft.T g1 in PSUM, then DVE adds the
    host-computed G_u rows during the PSUM->SBUF copy.
  - PSUM: z1 as 2x [128,1024] f32 tiles (tag psa, bufs=2) + a ring of
    [128,2,512] tiles (tag psb, bufs=2) = all 8 banks.
  - HAM warmup matmuls at start and at the phase boundary keep the PE
    clock gate at 8/8 through the ACT-bound stretches.
"""

import numpy as np
import ml_dtypes

B = 131072
S = 2
H = 512
E = 8
NCORES = 8
BC = B // NCORES   # 16384 samples per core
NSLICE = 512       # batch slice (matmul moving free dim)
NS = BC // NSLICE  # 32 slices
HC = H // 128      # 4 hidden-dim chunks of 128 partitions
LG = 2             # slices per x/a DMA load group

BF16 = ml_dtypes.bfloat16
F8 = ml_dtypes.float8_e4m3   # TRN FP8_EXP4 (bias 7, max 240)

W2S = 16.0         # fp8 scale on W2 (forward z2 matmul)
WUS = 4096.0       # fp8 scale on 0.5*W2*w3 (backward u matmul)

_cached = {}
last_results = None  # test.py introspects this for profiling info


def _pin_act_tables():
    """Restrict the activation-table chooser to the two sets this kernel
    wants (softplus; sigmoid+tanh) so Bacc's insert_act_table_loads
    doesn't ping-pong between sets. Set ids are positional, so unwanted
    sets are emptied, not removed."""
    import functools
    import concourse.hw_specs as hw_specs
    import concourse.bacc as bacc

    if getattr(hw_specs.get_activation_tables, "_ph_pinned", False):
        return
    orig = hw_specs.get_activation_tables
    KEEP = {"natural_log_exp_and_others", "sigmoid_and_others"}

    @functools.cache
    def pinned(module_arch):
        full = orig(module_arch)
        return {n: (f if n in KEEP else set()) for n, f in full.items()}

    pinned._ph_pinned = True
    hw_specs.get_activation_tables = pinned
    bacc.get_activation_tables = pinned


def _build_nc(b2_zero: bool):
    import concourse.bacc as bacc
    import concourse.mybir as mybir
    import concourse.tile as tile

    _pin_act_tables()

    f32 = mybir.dt.float32
    bf16 = mybir.dt.bfloat16
    f8 = mybir.dt.float8e4
    ADD = mybir.AluOpType.add
    MULT = mybir.AluOpType.mult
    EXP = mybir.ActivationFunctionType.Exp
    LN = mybir.ActivationFunctionType.Ln
    SIG = mybir.ActivationFunctionType.Sigmoid
    TANH = mybir.ActivationFunctionType.Tanh
    DR = mybir.MatmulPerfMode.DoubleRow

    nc = bacc.Bacc("TRN2", target_bir_lowering=False, debug=False)

    xT_d = nc.dram_tensor("xT", [7, BC], bf16, kind="ExternalInput")
    guT_d = nc.dram_tensor("guT", [S, BC], f32, kind="ExternalInput")
    # W1-aug row-tiled: rows 32j+r (r<7) hold [W1hi;W1hi;W1lo;b1][r, 128j:128j+128]
    w1rt_d = nc.dram_tensor("w1rt", [128, 128], bf16, kind="ExternalInput")
    # DoubleRow fp8 weights: [p, j2, pair, col]; logical k = j2*256 + pair*128 + p
    w2q_d = nc.dram_tensor("w2q", [128, 2, 2, H], f8, kind="ExternalInput")
    wu2_d = nc.dram_tensor("wu2", [128, 2, 2, H], f8, kind="ExternalInput")
    w1ft_d = nc.dram_tensor("w1ft", [128, HC, S], bf16, kind="ExternalInput")
    cv_d = nc.dram_tensor("cv", [128, HC], f32, kind="ExternalInput")
    b2h_d = nc.dram_tensor("b2h", [128, HC], f32, kind="ExternalInput")
    outT_d = nc.dram_tensor("outT", [S, BC], f32, kind="ExternalOutput")

    with tile.TileContext(nc) as tc:
        with (
            tc.tile_pool(name="consts", bufs=1) as consts,
            tc.tile_pool(name="work", bufs=2) as work,
            tc.tile_pool(name="ps", bufs=1, space="PSUM") as ps,
        ):
            # ---- constants ----
            # w1rt + the first x group load FIRST so the z1(0) critical path
            # isn't queued behind the big fp8 weight DMAs.
            w1rt = consts.tile([128, 128], bf16)
            nc.sync.dma_start(w1rt[:], w1rt_d[:])

            def load_x_rt(g, tag, split_first=False):
                """x load group replicated at partition offsets 0/32/64/96
                for row-tiled z1 matmuls.  split_first issues the first
                slice's quarter-loads ahead so z1(0) starts sooner."""
                x_t = work.tile(
                    [128, LG * NSLICE], bf16, tag="xa", bufs=2, name=f"x{tag}"
                )
                if split_first:
                    for sl in range(LG):
                        csl = slice(
                            (g * LG + sl) * NSLICE, (g * LG + sl + 1) * NSLICE
                        )
                        for j in range(4):
                            nc.sync.dma_start(
                                x_t[32 * j : 32 * j + 7,
                                    sl * NSLICE : (sl + 1) * NSLICE],
                                xT_d[:, csl],
                            )
                else:
                    csl = slice(g * LG * NSLICE, (g + 1) * LG * NSLICE)
                    for j in range(4):
                        nc.sync.dma_start(x_t[32 * j : 32 * j + 7, :], xT_d[:, csl])
                return x_t

            x_t0 = load_x_rt(0, "a0")

            w2q = consts.tile([128, 2, 2, H], f8)
            nc.sync.dma_start(w2q[:], w2q_d[:])
            wu2 = consts.tile([128, 2, 2, H], f8)
            nc.sync.dma_start(wu2[:], wu2_d[:])
            w1ft = consts.tile([128, HC, S], bf16)
            nc.sync.dma_start(w1ft[:], w1ft_d[:])
            cv = consts.tile([128, HC], f32)
            nc.sync.dma_start(cv[:], cv_d[:])
            b2h = consts.tile([128, HC], f32)
            nc.sync.dma_start(b2h[:], b2h_d[:])

            # z2 (bf16), persisted across the activation-table switch:
            # [partition, hidden-chunk, slice, col]
            z2s = consts.tile([128, HC, NS, NSLICE], bf16)

            def z1_matmuls(x_t, s, zpa, zpb):
                """4 K=7 matmuls packed into 4 concurrent PE row groups;
                chunk j lands in (zpa, zpb)[j // 2][:, (j % 2)*NSLICE:...].
                s = slice index within the x_t load group."""
                for j in range(4):
                    zp = zpa if j < 2 else zpb
                    nc.tensor.matmul(
                        zp[:, (j % 2) * NSLICE : (j % 2 + 1) * NSLICE],
                        w1rt[32 * j : 32 * j + 7, :],
                        x_t[32 * j : 32 * j + 7, s * NSLICE : (s + 1) * NSLICE],
                        start=True,
                        stop=True,
                        tile_position=(32 * j, 0),
                    )

            # HAM warmup: dummy matmuls fill the startup gap while the
            # weight DMAs stream in, forcing the PE clock gate to 8/8
            # before the first real matmul.
            warm = work.tile([128, NSLICE], bf16, tag="warm", bufs=1)
            nc.vector.memset(warm[:], 0.0)

            # ============ phase A: z1 -> softplus -> z2 (softplus table) ======
            # PE runs the z2 matmuls one slice behind ACT's softplus.
            h1_tiles = {}
            def z2_matmuls(s):
                h1 = h1_tiles.pop(s)
                for jj in range(2):
                    z2p = ps.tile(
                        [128, 2, NSLICE], f32, tag="psb", bufs=2,
                        name=f"z2p{s}_{jj}",
                    )
                    for ih in range(2):
                        ic = 2 * jj + ih
                        for j2 in range(2):
                            nc.tensor.matmul(
                                z2p[:, ih, :],
                                w2q[:, j2, :, ic * 128 : (ic + 1) * 128],
                                h1[:, 2 * j2 : 2 * j2 + 2, :],
                                start=(j2 == 0),
                                stop=(j2 == 1),
                                perf_mode=DR,
                            )
                    nc.vector.tensor_copy(
                        z2s[:, 2 * jj : 2 * jj + 2, s, :], z2p[:]
                    )

            for s in range(NS):
                if s % LG == 0:
                    x_t = x_t0 if s == 0 else load_x_rt(s // LG, f"a{s}")
                z1pa = ps.tile(
                    [128, 2 * NSLICE], f32, tag="psa", bufs=2, name=f"z1pa{s}"
                )
                z1pb = ps.tile(
                    [128, 2 * NSLICE], f32, tag="psa", bufs=2, name=f"z1pb{s}"
                )
                z1_matmuls(x_t, s % LG, z1pa, z1pb)
                e1a = work.tile(
                    [128, 2 * NSLICE], bf16, tag="e1", bufs=2, name=f"e1a_{s}"
                )
                e1b = work.tile(
                    [128, 2 * NSLICE], bf16, tag="e1", bufs=2, name=f"e1b_{s}"
                )
                nc.scalar.activation(e1a[:], z1pa[:], EXP)
                nc.scalar.activation(e1b[:], z1pb[:], EXP)
                h1 = work.tile(
                    [128, HC, NSLICE], f8, tag="h1", bufs=2, name=f"h1_{s}"
                )
                nc.scalar.activation(h1[:, 0:2, :], e1a[:], LN, bias=1.0)
                nc.scalar.activation(h1[:, 2:4, :], e1b[:], LN, bias=1.0)
                h1_tiles[s] = h1
                if s == 0:
                    # HAM warmup behind slice 0's ACT work: keeps the PE
                    # clock gate fed while the first exp/ln run.
                    wp = ps.tile(
                        [128, 2, NSLICE], f32, tag="psb", bufs=2, name="warmps"
                    )
                    for i in range(10):
                        nc.tensor.matmul(
                            wp[:, i % 2, :], warm[:, :128], warm[:],
                            start=True, stop=True, skip_group_check=True,
                        )
                if s > 0:
                    z2_matmuls(s - 1)
            z2_matmuls(NS - 1)

            # Fence: phase-B tanh reads its scale from an AP produced after
            # the LAST z2s write, so the scheduler cannot hoist any
            # sigmoid-set ACT instruction into phase A (table thrash).
            halfscale = consts.tile([128, 1], f32)
            nc.vector.tensor_scalar(
                halfscale[:], z2s[:, 0, NS - 1, 0:1], 0.0, 0.5 / W2S,
                mybir.AluOpType.mult, ADD,
            )

            # Re-warm the PE clock gate across the table-switch boundary:
            # phase A is ACT-bound and lets HAM re-throttle; phase B is
            # PE-heavy and needs the 2.4 GHz clock.
            for i in range(14):
                wp2 = ps.tile(
                    [128, 2, NSLICE], f32, tag="psb", bufs=2, name=f"rw{i}"
                )
                nc.tensor.matmul(
                    wp2[:, i % 2, :], warm[:, :128], warm[:], start=True,
                    stop=True, skip_group_check=True,
                )

            # ============ phase B: backward to output (sigmoid/tanh table) ====
            # out matmuls for a slice PAIR run column-tiled: the [2, n]
            # output uses 2 of 128 PE columns, so the even slice's matmuls
            # go to col-group 0 and the odd slice's to col-group 1 and they
            # execute concurrently in the array.
            t2qs = {}
            def t2blk(g):
                t2q = work.tile(
                    [128, HC, 4, NSLICE], f8, tag="t2", bufs=2, name=f"t2q{g}"
                )
                if b2_zero:
                    nc.scalar.activation(
                        t2q[:], z2s[:, :, 4 * g : 4 * g + 4, :], TANH,
                        scale=halfscale[:, 0:1],
                    )
                else:
                    for sl in range(4):
                        for ic in range(HC):
                            nc.scalar.activation(
                                t2q[:, ic, sl, :],
                                z2s[:, ic, 4 * g + sl, :], TANH,
                                bias=b2h[:, ic : ic + 1],
                                scale=halfscale[:, 0:1],
                            )
                t2qs[g] = t2q

            out_args = {}
            def out_matmuls(p):
                g1p, gu_t = out_args.pop(p)
                op = ps.tile(
                    [128, 2, NSLICE], f32, tag="psb", bufs=2, name=f"op{p}"
                )
                for kc in range(HC):
                    for half in range(2):
                        nc.tensor.matmul(
                            op[32 * half : 32 * half + S, 0, :],
                            w1ft[:, kc, :],
                            g1p[:, kc, half, :],
                            start=(kc == 0),
                            stop=(kc == HC - 1),
                            skip_group_check=True,
                            tile_position=(0, 32 * half),
                        )
                o_t = work.tile(
                    [S, 2 * NSLICE], f32, tag="osb", bufs=2, name=f"ot{p}"
                )
                nc.vector.tensor_tensor(
                    o_t[:, 0:NSLICE], op[0:S, 0, :], gu_t[:, 0:NSLICE], ADD
                )
                nc.vector.tensor_tensor(
                    o_t[:, NSLICE:], op[32 : 32 + S, 0, :], gu_t[:, NSLICE:], ADD
                )
                nc.sync.dma_start(
                    outT_d[:, 2 * p * NSLICE : (2 * p + 2) * NSLICE], o_t[:]
                )

            for s in range(NS):
                if s % LG == 0:
                    x_t = load_x_rt(s // LG, f"b{s}")
                    gu_t = work.tile(
                        [S, LG * NSLICE], f32, tag="aa", bufs=2, name=f"gu{s}"
                    )
                    nc.sync.dma_start(
                        gu_t[:], guT_d[:, s * NSLICE : (s + LG) * NSLICE]
                    )

                if s == 0:
                    t2blk(0)
                z1qa = ps.tile(
                    [128, 2 * NSLICE], f32, tag="psa", bufs=2, name=f"z1qa{s}"
                )
                z1qb = ps.tile(
                    [128, 2 * NSLICE], f32, tag="psa", bufs=2, name=f"z1qb{s}"
                )
                z1_matmuls(x_t, s % LG, z1qa, z1qb)
                sg1 = work.tile(
                    [128, HC, NSLICE], bf16, tag="sg1", bufs=2, name=f"sg1_{s}"
                )
                nc.scalar.activation(sg1[:, 0:2, :], z1qa[:], SIG)
                nc.scalar.activation(sg1[:, 2:4, :], z1qb[:], SIG)
                if s % 4 == 2 and s // 4 + 1 < NS // 4:
                    t2blk(s // 4 + 1)

                if s % 2 == 0:
                    g1p = work.tile(
                        [128, HC, 2, NSLICE], bf16, tag="g1", bufs=2,
                        name=f"g1_{s}",
                    )
                for jj in range(2):
                    up = ps.tile(
                        [128, 2, NSLICE], f32, tag="psb", bufs=2,
                        name=f"up{s}_{jj}",
                    )
                    for ih in range(2):
                        jc = 2 * jj + ih
                        for j2 in range(2):
                            nc.tensor.matmul(
                                up[:, ih, :],
                                wu2[:, j2, :, jc * 128 : (jc + 1) * 128],
                                t2qs[s // 4][:, 2 * j2 : 2 * j2 + 2, s % 4, :],
                                start=(j2 == 0),
                                stop=(j2 == 1),
                                perf_mode=DR,
                            )
                        nc.vector.scalar_tensor_tensor(
                            g1p[:, jc, s % 2, :],
                            up[:, ih, :],
                            cv[:, jc : jc + 1],
                            sg1[:, jc, :],
                            ADD,
                            MULT,
                        )
                if s % 2 == 1:
                    out_args[s // 2] = (g1p, gu_t)
                    if s >= 3:
                        out_matmuls(s // 2 - 1)
            out_matmuls(NS // 2 - 1)

    nc.compile()
    return nc


def _hi_lo(a32):
    hi = a32.astype(BF16)
    lo = (a32 - hi.astype(np.float32)).astype(BF16)
    return hi, lo


def _dr_pack(w, scale):
    """[512, 512] f32 -> [128, 2, 2, 512] fp8 DoubleRow weight layout:
    out[p, j2, i, col] = w[j2*256 + i*128 + p, col] * scale."""
    return np.ascontiguousarray(
        (w * scale).reshape(2, 2, 128, H).transpose(2, 0, 1, 3)
    ).astype(F8)


def kernel(
    t,
    state,
    action_emb,
    W1,
    b1,
    W2,
    b2,
    W3,
    b3,
    log_damping,
    Gw,
    Gb,
):
    global last_results
    import os
    from concourse.bass_utils import run_bass_kernel_spmd

    state = np.asarray(state, dtype=np.float32)
    action_emb = np.asarray(action_emb, dtype=np.float32)
    W1 = np.asarray(W1, dtype=np.float32)
    b1 = np.asarray(b1, dtype=np.float32)
    W2 = np.asarray(W2, dtype=np.float32)
    b2 = np.asarray(b2, dtype=np.float32)
    W3 = np.asarray(W3, dtype=np.float32)
    b3 = np.asarray(b3, dtype=np.float32)  # unused: constant shift, no grad
    damping = float(np.exp(np.float32(log_damping)))
    Gw = np.asarray(Gw, dtype=np.float32)
    Gb = np.asarray(Gb, dtype=np.float32)

    # ---- host-side weight prep (tiny) ----
    w3col = W3[:, 0]
    w1hi, w1lo = _hi_lo(W1)  # [2, H] each
    w1a = np.concatenate(
        [w1hi, w1hi, w1lo, b1[None, :].astype(BF16)], axis=0
    )  # [7, H] bf16
    # row-tiled layout: rows 32j+r = w1a[r, 128j:128j+128]
    w1rt = np.zeros((128, 128), dtype=BF16)
    for j in range(4):
        w1rt[32 * j : 32 * j + 7, :] = w1a[:, 128 * j : 128 * (j + 1)]

    w2q = _dr_pack(W2, W2S)                       # forward z2 weights
    wu_raw = (W2 * w3col[None, :]).T              # [i, j] = W2[j,i]*w3[i]
    wu2 = _dr_pack(wu_raw, 0.5 * WUS)             # backward u weights
    # c_j = 0.5*WUS*sum_i wu_raw[i,j], per-partition-per-chunk [128, HC]
    cvec = 0.5 * WUS * wu_raw.sum(axis=0)
    cv = np.ascontiguousarray(cvec.reshape(HC, 128).T).astype(np.float32)

    M = np.array([[0.0, 1.0], [-1.0, -damping]], dtype=np.float32)
    w1f = (M @ W1) / WUS  # [2, H]
    w1ftr = w1f.T.astype(BF16).reshape(HC, 128, S).transpose(1, 0, 2).copy()

    b2h = np.ascontiguousarray((b2 * 0.5).reshape(HC, 128).T).astype(np.float32)
    b2_zero = not np.any(b2)

    # ---- per-core input shards ----
    sT = state.T  # [2, B]
    shi, slo = _hi_lo(sT)
    ones_row = np.ones((1, B), dtype=BF16)
    xT = np.concatenate([shi, slo, shi, ones_row], axis=0)  # [7, B]

    # G_u computed on host (tiny matvec); row 0 zero so the DVE add is a
    # single [2, n] tensor_tensor against the out PSUM.
    gu = action_emb @ Gw + Gb[None, :]  # [B, 1]
    guT = np.concatenate(
        [np.zeros((1, B), np.float32), gu.T.astype(np.float32)], axis=0
    )  # [2, B]

    key = ("nc", b2_zero)
    if key not in _cached:
        _cached[key] = _build_nc(b2_zero)
    nc = _cached[key]

    in_maps = []
    for c in range(NCORES):
        csl = slice(c * BC, (c + 1) * BC)
        in_maps.append(
            {
                "xT": np.ascontiguousarray(xT[:, csl]),
                "guT": np.ascontiguousarray(guT[:, csl]),
                "w1rt": w1rt,
                "w2q": w2q,
                "wu2": wu2,
                "w1ft": w1ftr,
                "cv": cv,
                "b2h": b2h,
            }
        )

    trace = bool(os.environ.get("PH_TRACE"))
    res = run_bass_kernel_spmd(
        nc, in_maps, core_ids=list(range(NCORES)), trace=trace
    )
    last_results = res

    out = np.empty((B, S), dtype=np.float32)
    for c in range(NCORES):
        out[c * BC : (c + 1) * BC, :] = res.results[c]["outT"].T
    return out


# revision 40
# speedup vs baseline: 1.1279x; 1.1279x over previous
"""Port-Hamiltonian model forward pass (dstate/dt) as a Bass/Tile kernel on
8 TRN2 NeuronCores, pure data-parallel over the batch.

Math (per sample, feature-major / transposed layout on chip):
    z1T = W1.T sT + b1                  [512, n]
    h1T = softplus(z1T) = Ln(Exp(z1T)+1)  (fp8 out on the Ln)
    z2T = W2.T h1T                      (fp8 DoubleRow matmuls, x W2S)
    t2  = tanh((z2T + b2)/2)            (fp8 out; sigma(z2) = (1+t2)/2)
    uT  = (0.5 W2 w3).T t2 + c          c_j = 0.5 sum_i W2[j,i] w3[i]
    g1T = (uT + c) * sigmoid(z1T)       one DVE scalar_tensor_tensor per chunk
    outT = (M @ W1) g1T + [0; G_u]      G_u = action_emb @ Gw + Gb on host
  where M = [[0, 1], [-1, -damping]].

Two ACT table phases per core (exp/ln set, then sigmoid/tanh set); the
big H x H matmuls (z2 forward, u backward) run as fp8e4 DoubleRow pairs
(K=256 per matmul, 2x PE throughput).  Weight scales W2S/WUS keep fp8
operands in the normal range; the tanh centering removes the sigma-mean
bulk from the backward operand so fp8 quantization noise halves.

Structure per core (16384 samples, 32 slices of 512):
  - phase A (exp/ln table): z1 (4 concurrent row-tiled K=7 matmuls)
    -> exp -> ln(x+1) -> h1 fp8 -> z2 (8 DoubleRow MMs) -> z2 stored
    bf16 in SBUF by DVE.  PE pipelined one slice behind ACT.
  - phase B (sigmoid table): t2 = tanh(z2s/2) fp8 (scale rides an AP
    written after the last z2s store, fencing the table switch), z1
    recomputed, sg1 = sigmoid(z1), u (8 DoubleRow MMs), g1 = (u+c)*sg1
    fused on DVE, out = w1ft.T g1 in PSUM, then DVE adds the
    host-computed G_u rows during the PSUM->SBUF copy.
  - PSUM: z1 as 2x [128,1024] f32 tiles (tag psa, bufs=2) + a ring of
    [128,2,512] tiles (tag psb, bufs=2) = all 8 banks.
  - HAM warmup matmuls at start and at the phase boundary keep the PE
    clock gate at 8/8 through the ACT-bound stretches.
"""

import numpy as np
import ml_dtypes

B = 131072
S = 2
H = 512
E = 8
NCORES = 8
BC = B // NCORES   # 16384 samples per core
NSLICE = 512       # batch slice (matmul moving free dim)
NS = BC // NSLICE  # 32 slices
HC = H // 128      # 4 hidden-dim chunks of 128 partitions
LG = 2             # slices per x/a DMA load group

BF16 = ml_dtypes.bfloat16
F8 = ml_dtypes.float8_e4m3   # TRN FP8_EXP4 (bias 7, max 240)

W2S = 16.0         # fp8 scale on W2 (forward z2 matmul)
WUS = 4096.0       # fp8 scale on 0.5*W2*w3 (backward u matmul)

_cached = {}
last_results = None  # test.py introspects this for profiling info


def _pin_act_tables():
    """Restrict the activation-table chooser to the two sets this kernel
    wants (softplus; sigmoid+tanh) so Bacc's insert_act_table_loads
    doesn't ping-pong between sets. Set ids are positional, so unwanted
    sets are emptied, not removed."""
    import functools
    import concourse.hw_specs as hw_specs
    import concourse.bacc as bacc

    if getattr(hw_specs.get_activation_tables, "_ph_pinned", False):
        return
    orig = hw_specs.get_activation_tables
    KEEP = {"natural_log_exp_and_others", "sigmoid_and_others"}

    @functools.cache
    def pinned(module_arch):
        full = orig(module_arch)
        return {n: (f if n in KEEP else set()) for n, f in full.items()}

    pinned._ph_pinned = True
    hw_specs.get_activation_tables = pinned
    bacc.get_activation_tables = pinned


def _build_nc(b2_zero: bool):
    import concourse.bacc as bacc
    import concourse.mybir as mybir
    import concourse.tile as tile

    _pin_act_tables()

    f32 = mybir.dt.float32
    bf16 = mybir.dt.bfloat16
    f8 = mybir.dt.float8e4
    ADD = mybir.AluOpType.add
    MULT = mybir.AluOpType.mult
    EXP = mybir.ActivationFunctionType.Exp
    LN = mybir.ActivationFunctionType.Ln
    SIG = mybir.ActivationFunctionType.Sigmoid
    TANH = mybir.ActivationFunctionType.Tanh
    DR = mybir.MatmulPerfMode.DoubleRow

    nc = bacc.Bacc("TRN2", target_bir_lowering=False, debug=False)

    xT_d = nc.dram_tensor("xT", [7, BC], bf16, kind="ExternalInput")
    guT_d = nc.dram_tensor("guT", [S, BC], f32, kind="ExternalInput")
    # W1-aug row-tiled: rows 32j+r (r<7) hold [W1hi;W1hi;W1lo;b1][r, 128j:128j+128]
    w1rt_d = nc.dram_tensor("w1rt", [128, 128], bf16, kind="ExternalInput")
    # DoubleRow fp8 weights: [p, j2, pair, col]; logical k = j2*256 + pair*128 + p
    w2q_d = nc.dram_tensor("w2q", [128, 2, 2, H], f8, kind="ExternalInput")
    wu2_d = nc.dram_tensor("wu2", [128, 2, 2, H], f8, kind="ExternalInput")
    w1ft_d = nc.dram_tensor("w1ft", [128, HC, S], bf16, kind="ExternalInput")
    cv_d = nc.dram_tensor("cv", [128, HC], f32, kind="ExternalInput")
    b2h_d = nc.dram_tensor("b2h", [128, HC], f32, kind="ExternalInput")
    outT_d = nc.dram_tensor("outT", [S, BC], f32, kind="ExternalOutput")

    with tile.TileContext(nc) as tc:
        with (
            tc.tile_pool(name="consts", bufs=1) as consts,
            tc.tile_pool(name="work", bufs=2) as work,
            tc.tile_pool(name="ps", bufs=1, space="PSUM") as ps,
        ):
            # ---- constants ----
            # w1rt + the first x group load FIRST so the z1(0) critical path
            # isn't queued behind the big fp8 weight DMAs.
            w1rt = consts.tile([128, 128], bf16)
            nc.sync.dma_start(w1rt[:], w1rt_d[:])

            def load_x_rt(g, tag, split_first=False):
                """x load group replicated at partition offsets 0/32/64/96
                for row-tiled z1 matmuls.  split_first issues the first
                slice's quarter-loads ahead so z1(0) starts sooner."""
                x_t = work.tile(
                    [128, LG * NSLICE], bf16, tag="xa", bufs=2, name=f"x{tag}"
                )
                if split_first:
                    for sl in range(LG):
                        csl = slice(
                            (g * LG + sl) * NSLICE, (g * LG + sl + 1) * NSLICE
                        )
                        for j in range(4):
                            nc.sync.dma_start(
                                x_t[32 * j : 32 * j + 7,
                                    sl * NSLICE : (sl + 1) * NSLICE],
                                xT_d[:, csl],
                            )
                else:
                    csl = slice(g * LG * NSLICE, (g + 1) * LG * NSLICE)
                    for j in range(4):
                        nc.sync.dma_start(x_t[32 * j : 32 * j + 7, :], xT_d[:, csl])
                return x_t

            x_t0 = load_x_rt(0, "a0")

            w2q = consts.tile([128, 2, 2, H], f8)
            nc.sync.dma_start(w2q[:], w2q_d[:])
            wu2 = consts.tile([128, 2, 2, H], f8)
            nc.sync.dma_start(wu2[:], wu2_d[:])
            w1ft = consts.tile([128, HC, S], bf16)
            nc.sync.dma_start(w1ft[:], w1ft_d[:])
            cv = consts.tile([128, HC], f32)
            nc.sync.dma_start(cv[:], cv_d[:])
            b2h = consts.tile([128, HC], f32)
            nc.sync.dma_start(b2h[:], b2h_d[:])

            # z2 (bf16), persisted across the activation-table switch:
            # [partition, hidden-chunk, slice, col]
            z2s = consts.tile([128, HC, NS, NSLICE], bf16)

            def z1_matmuls(x_t, s, zpa, zpb):
                """4 K=7 matmuls packed into 4 concurrent PE row groups;
                chunk j lands in (zpa, zpb)[j // 2][:, (j % 2)*NSLICE:...].
                s = slice index within the x_t load group."""
                for j in range(4):
                    zp = zpa if j < 2 else zpb
                    nc.tensor.matmul(
                        zp[:, (j % 2) * NSLICE : (j % 2 + 1) * NSLICE],
                        w1rt[32 * j : 32 * j + 7, :],
                        x_t[32 * j : 32 * j + 7, s * NSLICE : (s + 1) * NSLICE],
                        start=True,
                        stop=True,
                        tile_position=(32 * j, 0),
                    )

            # HAM warmup: dummy matmuls fill the startup gap while the
            # weight DMAs stream in, forcing the PE clock gate to 8/8
            # before the first real matmul.
            warm = work.tile([128, NSLICE], bf16, tag="warm", bufs=1)
            nc.vector.memset(warm[:], 0.0)

            # ============ phase A: z1 -> softplus -> z2 (softplus table) ======
            # PE runs the z2 matmuls one slice behind ACT's softplus.
            h1_tiles = {}
            def z2_matmuls(s):
                h1 = h1_tiles.pop(s)
                for jj in range(2):
                    z2p = ps.tile(
                        [128, 2, NSLICE], f32, tag="psb", bufs=2,
                        name=f"z2p{s}_{jj}",
                    )
                    for ih in range(2):
                        ic = 2 * jj + ih
                        for j2 in range(2):
                            nc.tensor.matmul(
                                z2p[:, ih, :],
                                w2q[:, j2, :, ic * 128 : (ic + 1) * 128],
                                h1[:, 2 * j2 : 2 * j2 + 2, :],
                                start=(j2 == 0),
                                stop=(j2 == 1),
                                perf_mode=DR,
                            )
                    nc.vector.tensor_copy(
                        z2s[:, 2 * jj : 2 * jj + 2, s, :], z2p[:]
                    )

            for s in range(NS):
                if s % LG == 0:
                    x_t = x_t0 if s == 0 else load_x_rt(s // LG, f"a{s}")
                z1pa = ps.tile(
                    [128, 2 * NSLICE], f32, tag="psa", bufs=2, name=f"z1pa{s}"
                )
                z1pb = ps.tile(
                    [128, 2 * NSLICE], f32, tag="psa", bufs=2, name=f"z1pb{s}"
                )
                z1_matmuls(x_t, s % LG, z1pa, z1pb)
                e1a = work.tile(
                    [128, 2 * NSLICE], bf16, tag="e1", bufs=2, name=f"e1a_{s}"
                )
                e1b = work.tile(
                    [128, 2 * NSLICE], bf16, tag="e1", bufs=2, name=f"e1b_{s}"
                )
                nc.scalar.activation(e1a[:], z1pa[:], EXP)
                nc.scalar.activation(e1b[:], z1pb[:], EXP)
                h1 = work.tile(
                    [128, HC, NSLICE], f8, tag="h1", bufs=3, name=f"h1_{s}"
                )
                nc.scalar.activation(h1[:, 0:2, :], e1a[:], LN, bias=1.0)
                nc.scalar.activation(h1[:, 2:4, :], e1b[:], LN, bias=1.0)
                h1_tiles[s] = h1
                if s == 0:
                    # HAM warmup behind slice 0's ACT work: keeps the PE
                    # clock gate fed while the first exp/ln run.
                    wp = ps.tile(
                        [128, 2, NSLICE], f32, tag="psb", bufs=2, name="warmps"
                    )
                    for i in range(10):
                        nc.tensor.matmul(
                            wp[:, i % 2, :], warm[:, :128], warm[:],
                            start=True, stop=True, skip_group_check=True,
                        )
                if s > 0:
                    z2_matmuls(s - 1)
            z2_matmuls(NS - 1)

            # Fence: phase-B tanh reads its scale from an AP produced after
            # the LAST z2s write, so the scheduler cannot hoist any
            # sigmoid-set ACT instruction into phase A (table thrash).
            halfscale = consts.tile([128, 1], f32)
            nc.vector.tensor_scalar(
                halfscale[:], z2s[:, 0, NS - 1, 0:1], 0.0, 0.5 / W2S,
                mybir.AluOpType.mult, ADD,
            )

            # Re-warm the PE clock gate across the table-switch boundary:
            # phase A is ACT-bound and lets HAM re-throttle; phase B is
            # PE-heavy and needs the 2.4 GHz clock.
            for i in range(14):
                wp2 = ps.tile(
                    [128, 2, NSLICE], f32, tag="psb", bufs=2, name=f"rw{i}"
                )
                nc.tensor.matmul(
                    wp2[:, i % 2, :], warm[:, :128], warm[:], start=True,
                    stop=True, skip_group_check=True,
                )

            # ============ phase B: backward to output (sigmoid/tanh table) ====
            # out matmuls for a slice PAIR run column-tiled: the [2, n]
            # output uses 2 of 128 PE columns, so the even slice's matmuls
            # go to col-group 0 and the odd slice's to col-group 1 and they
            # execute concurrently in the array.
            out_args = {}
            def out_matmuls(p):
                g1p, gu_t = out_args.pop(p)
                op = ps.tile(
                    [128, 2, NSLICE], f32, tag="psb", bufs=2, name=f"op{p}"
                )
                for kc in range(HC):
                    for half in range(2):
                        nc.tensor.matmul(
                            op[32 * half : 32 * half + S, 0, :],
                            w1ft[:, kc, :],
                            g1p[:, kc, half, :],
                            start=(kc == 0),
                            stop=(kc == HC - 1),
                            skip_group_check=True,
                            tile_position=(0, 32 * half),
                        )
                o_t = work.tile(
                    [S, 2 * NSLICE], f32, tag="osb", bufs=2, name=f"ot{p}"
                )
                nc.vector.tensor_tensor(
                    o_t[:, 0:NSLICE], op[0:S, 0, :], gu_t[:, 0:NSLICE], ADD
                )
                nc.vector.tensor_tensor(
                    o_t[:, NSLICE:], op[32 : 32 + S, 0, :], gu_t[:, NSLICE:], ADD
                )
                nc.sync.dma_start(
                    outT_d[:, 2 * p * NSLICE : (2 * p + 2) * NSLICE], o_t[:]
                )

            for s in range(NS):
                if s % LG == 0:
                    x_t = load_x_rt(s // LG, f"b{s}")
                    gu_t = work.tile(
                        [S, LG * NSLICE], f32, tag="aa", bufs=2, name=f"gu{s}"
                    )
                    nc.sync.dma_start(
                        gu_t[:], guT_d[:, s * NSLICE : (s + LG) * NSLICE]
                    )

                t2 = work.tile(
                    [128, HC, NSLICE], f8, tag="t2", bufs=3, name=f"t2_{s}"
                )
                if b2_zero:
                    nc.scalar.activation(
                        t2[:], z2s[:, :, s, :], TANH, scale=halfscale[:, 0:1]
                    )
                else:
                    for ic in range(HC):
                        nc.scalar.activation(
                            t2[:, ic, :], z2s[:, ic, s, :], TANH,
                            bias=b2h[:, ic : ic + 1], scale=halfscale[:, 0:1],
                        )

                z1qa = ps.tile(
                    [128, 2 * NSLICE], f32, tag="psa", bufs=2, name=f"z1qa{s}"
                )
                z1qb = ps.tile(
                    [128, 2 * NSLICE], f32, tag="psa", bufs=2, name=f"z1qb{s}"
                )
                z1_matmuls(x_t, s % LG, z1qa, z1qb)
                sg1 = work.tile(
                    [128, HC, NSLICE], bf16, tag="sg1", bufs=3, name=f"sg1_{s}"
                )
                nc.scalar.activation(sg1[:, 0:2, :], z1qa[:], SIG)
                nc.scalar.activation(sg1[:, 2:4, :], z1qb[:], SIG)

                if s % 2 == 0:
                    g1p = work.tile(
                        [128, HC, 2, NSLICE], bf16, tag="g1", bufs=2,
                        name=f"g1_{s}",
                    )
                for jj in range(2):
                    up = ps.tile(
                        [128, 2, NSLICE], f32, tag="psb", bufs=2,
                        name=f"up{s}_{jj}",
                    )
                    for ih in range(2):
                        jc = 2 * jj + ih
                        for j2 in range(2):
                            nc.tensor.matmul(
                                up[:, ih, :],
                                wu2[:, j2, :, jc * 128 : (jc + 1) * 128],
                                t2[:, 2 * j2 : 2 * j2 + 2, :],
                                start=(j2 == 0),
                                stop=(j2 == 1),
                                perf_mode=DR,
                            )
                        nc.vector.scalar_tensor_tensor(
                            g1p[:, jc, s % 2, :],
                            up[:, ih, :],
                            cv[:, jc : jc + 1],
                            sg1[:, jc, :],
                            ADD,
                            MULT,
                        )
                if s % 2 == 1:
                    out_args[s // 2] = (g1p, gu_t)
                    if s >= 3:
                        out_matmuls(s // 2 - 1)
            out_matmuls(NS // 2 - 1)

    nc.compile()
    return nc


def _hi_lo(a32):
    hi = a32.astype(BF16)
    lo = (a32 - hi.astype(np.float32)).astype(BF16)
    return hi, lo


def _dr_pack(w, scale):
    """[512, 512] f32 -> [128, 2, 2, 512] fp8 DoubleRow weight layout:
    out[p, j2, i, col] = w[j2*256 + i*128 + p, col] * scale."""
    return np.ascontiguousarray(
        (w * scale).reshape(2, 2, 128, H).transpose(2, 0, 1, 3)
    ).astype(F8)


def kernel(
    t,
    state,
    action_emb,
    W1,
    b1,
    W2,
    b2,
    W3,
    b3,
    log_damping,
    Gw,
    Gb,
):
    global last_results
    import os
    from concourse.bass_utils import run_bass_kernel_spmd

    state = np.asarray(state, dtype=np.float32)
    action_emb = np.asarray(action_emb, dtype=np.float32)
    W1 = np.asarray(W1, dtype=np.float32)
    b1 = np.asarray(b1, dtype=np.float32)
    W2 = np.asarray(W2, dtype=np.float32)
    b2 = np.asarray(b2, dtype=np.float32)
    W3 = np.asarray(W3, dtype=np.float32)
    b3 = np.asarray(b3, dtype=np.float32)  # unused: constant shift, no grad
    damping = float(np.exp(np.float32(log_damping)))
    Gw = np.asarray(Gw, dtype=np.float32)
    Gb = np.asarray(Gb, dtype=np.float32)

    # ---- host-side weight prep (tiny) ----
    w3col = W3[:, 0]
    w1hi, w1lo = _hi_lo(W1)  # [2, H] each
    w1a = np.concatenate(
        [w1hi, w1hi, w1lo, b1[None, :].astype(BF16)], axis=0
    )  # [7, H] bf16
    # row-tiled layout: rows 32j+r = w1a[r, 128j:128j+128]
    w1rt = np.zeros((128, 128), dtype=BF16)
    for j in range(4):
        w1rt[32 * j : 32 * j + 7, :] = w1a[:, 128 * j : 128 * (j + 1)]

    w2q = _dr_pack(W2, W2S)                       # forward z2 weights
    wu_raw = (W2 * w3col[None, :]).T              # [i, j] = W2[j,i]*w3[i]
    wu2 = _dr_pack(wu_raw, 0.5 * WUS)             # backward u weights
    # c_j = 0.5*WUS*sum_i wu_raw[i,j], per-partition-per-chunk [128, HC]
    cvec = 0.5 * WUS * wu_raw.sum(axis=0)
    cv = np.ascontiguousarray(cvec.reshape(HC, 128).T).astype(np.float32)

    M = np.array([[0.0, 1.0], [-1.0, -damping]], dtype=np.float32)
    w1f = (M @ W1) / WUS  # [2, H]
    w1ftr = w1f.T.astype(BF16).reshape(HC, 128, S).transpose(1, 0, 2).copy()

    b2h = np.ascontiguousarray((b2 * 0.5).reshape(HC, 128).T).astype(np.float32)
    b2_zero = not np.any(b2)

    # ---- per-core input shards ----
    sT = state.T  # [2, B]
    shi, slo = _hi_lo(sT)
    ones_row = np.ones((1, B), dtype=BF16)
    xT = np.concatenate([shi, slo, shi, ones_row], axis=0)  # [7, B]

    # G_u computed on host (tiny matvec); row 0 zero so the DVE add is a
    # single [2, n] tensor_tensor against the out PSUM.
    gu = action_emb @ Gw + Gb[None, :]  # [B, 1]
    guT = np.concatenate(
        [np.zeros((1, B), np.float32), gu.T.astype(np.float32)], axis=0
    )  # [2, B]

    key = ("nc", b2_zero)
    if key not in _cached:
        _cached[key] = _build_nc(b2_zero)
    nc = _cached[key]

    in_maps = []
    for c in range(NCORES):
        csl = slice(c * BC, (c + 1) * BC)
        in_maps.append(
            {
                "xT": np.ascontiguousarray(xT[:, csl]),
                "guT": np.ascontiguousarray(guT[:, csl]),
                "w1rt": w1rt,
                "w2q": w2q,
                "wu2": wu2,
                "w1ft": w1ftr,
                "cv": cv,
                "b2h": b2h,
            }
        )

    trace = bool(os.environ.get("PH_TRACE"))
    res = run_bass_kernel_spmd(
        nc, in_maps, core_ids=list(range(NCORES)), trace=trace
    )
    last_results = res

    out = np.empty((B, S), dtype=np.float32)
    for c in range(NCORES):
        out[c * BC : (c + 1) * BC, :] = res.results[c]["outT"].T
    return out
